# revision 2
# baseline (speedup 1.0000x reference)
"""Trainium2 Bass kernel for nn_Encoder GNN message passing (8 NeuronCores).

Decomposition (dst-sharded, transposed compute):
  - nodes assigned to (core, sub-block<=32 nodes, pos) slots; each sub-block
    has <=512 in-edges, split into an A-stream (table rows < 32768) and a
    B-stream (rows >= 20480, indexed relative to 20480) of 2x128-edge chunks
    each, so indices fit dma_gather's int16.
  - per chunk: P^T[j, t*32+pos] += m_g[e, j]^T @ onehot[e, t*32+pos]
  - transform: agg^T[:, sub] = sum_t (2*W_t) @ P^T[:, t*32:(t+1)*32]
    (x2 because the reference duplicates every edge)
  - GRU (h == x0 always) computed transposed per 256-slot half-group with
    batched [128, 256]-wide gate matmuls and elementwise ops.
  - m table [NSLOTS, 128] bf16 in DRAM (double-buffered across steps),
    AllGather'd across cores per step.

Gather cost note: dma_gather descriptor generation on the Q7 runs at
~7.9ns/idx and scales with the STATIC num_idxs, so sub packing targets
nsub=200 (25 gather-quanta/step) instead of 208 (26).
"""
import sys
import types
import numpy as np
import ml_dtypes

import concourse.bass as bass
import concourse.mybir as mybir
import concourse.tile as tile
import concourse.tile_sem_assignment as _tsa
from concourse import library_config
from concourse.bass_utils import run_bass_kernel_spmd

# cap DMA completion-sem lanes so sync-wait splitting stays manageable
_tsa.NUM_SWDGE_GLOBAL_SEMS = 2
_tsa.NUM_HWDGE_SEMS = 2

# The tile scheduler's cost model thinks SWDGE descriptor generation runs at
# 0.34ns/desc; dma_gather ucode measures ~7.9ns/idx on HW. With the default
# the scheduler believes gathers are ~2.4us (actual ~32us) and sinks the
# interleaved AllGather pieces to the end of each step. Patch the model so
# scheduling decisions match reality (no semantic effect).
from concourse import hw_specs as _hw_specs
_hw_specs.TRN2Spec.SWDGE_NS_PER_DESCRIPTOR = 0.34

N, IN, L, STEPS, T, E = 50000, 64, 128, 3, 4, 800000
NCORES = 8
SUB_CAP_N = 32
SUB_CAP_E = 512           # 4 chunks x 128
NODES_PER_CORE = N // NCORES
B_BASE = 20480            # B-stream table base row
A_MAX = 32768             # A-stream rows must be < 32768
PAD_COL = 200.0           # one-hot sentinel (never equals iota 0..127)

F32, BF16, I16 = mybir.dt.float32, mybir.dt.bfloat16, mybir.dt.int16
BF = ml_dtypes.bfloat16


# ---------------------------------------------------------------- waitfix --
def _fix_sync_waits(nc):
    """This walrus accepts only ONE semaphore wait per instruction; move
    excess waits onto preceding same-engine NoOps (engine queues are
    in-order, so semantics are preserved)."""
    uid = [0]

    def mknop(engine, waits, debug):
        uid[0] += 1
        return mybir.InstNoOp(
            name=f"WFIX-{uid[0]}", engine=engine, ins=[], outs=[], debug=debug,
            sync_info=mybir.SyncInfo(on_wait=list(waits), on_update=[]))

    total = 0
    for bb in nc.main_func.blocks:
        il = bb.instructions
        i = 0
        while i < len(il):
            inst = il[i]
            si = inst.sync_info
            waits = list(si.on_wait) if si is not None else []
            if len(waits) > 1:
                inst.sync_info = mybir.SyncInfo(
                    on_wait=waits[:1], on_update=list(si.on_update))
                nops = [mknop(inst.engine, [w], inst.debug)
                        for w in waits[1:]]
                for k, nop in enumerate(nops):
                    il.insert(i + k, nop)
                    nc.register_instruction(nop, overwrite=True)
                i += len(nops)
                total += len(nops)
            i += 1
    return total


def _install_ntff_hook():
    if "antenv.axon_hooks" in sys.modules:
        return
    try:
        from trn_agent_boot.trn_boot import _ntff_profile_via_ctypes
        hook = _ntff_profile_via_ctypes("/opt/axon/libaxon_pjrt.so")
    except Exception:
        hook = None
    mod = types.ModuleType("antenv.axon_hooks")
    mod.get_axon_ntff_profile_hook = lambda: hook
    mod.set_axon_ntff_profile_hook = lambda h: None
    sys.modules["antenv.axon_hooks"] = mod
    import concourse.bass_utils as bu
    bu.upload_artifacts = lambda d: f"local:{d}"


def _pieces_of(groups):
    ng = len(groups)
    a, b = (5 * ng) // 13, (9 * ng) // 13
    pieces = [(0, a), (a, b), (b, ng)]
    return [(p0, p1) for p0, p1 in pieces if p1 > p0]


def _groups_of(nsub):
    """Split nsub sub-blocks into gather groups of <=16 subs (multiple of 4)."""
    gs, rem = [], nsub
    while rem > 0:
        take = min(16, rem)
        gs.append(take)
        rem -= take
    return gs


# ---------------------------------------------------------- preprocessing --
def _preprocess(edge_index, edge_attr):
    src = np.asarray(edge_index[0], np.int64)
    dst = np.asarray(edge_index[1], np.int64)
    bond = np.asarray(np.argmax(np.asarray(edge_attr), axis=1), np.int64)
    degsum = np.bincount(dst, minlength=N)

    # stage 1: nodes -> cores (balance total degree, exact node count)
    order = np.argsort(-degsum, kind="stable")
    node_core = np.full(N, -1, np.int64)
    core_load = np.zeros(NCORES, np.int64)
    core_count = np.zeros(NCORES, np.int64)
    for n in order:
        cand = np.flatnonzero(core_count < NODES_PER_CORE)
        c = cand[np.argmin(core_load[cand])]
        node_core[n] = c
        core_load[c] += degsum[n]
        core_count[c] += 1

    # stage 2: per-core packing into nsub bins (<=32 nodes, <=512 edges).
    # Start tight (200 bins -> 25 gather quanta/step) and relax if packing
    # or the A/B stream split fails.
    nsub0 = max(int(np.ceil(NODES_PER_CORE / SUB_CAP_N)),
                int(np.ceil(core_load.max() / SUB_CAP_E)))
    nsub0 = -(-nsub0 // 4) * 4
    import os as _os
    if _os.environ.get("FORCE_NSUB"):
        nsub0 = int(_os.environ["FORCE_NSUB"])
    nsub = nsub0
    packs = None
    for _attempt in range(8):
        packs, ok = [], True
        soft_e = min(SUB_CAP_E - 2, core_load.max() / nsub * 1.02)
        soft_n = SUB_CAP_N - 0.5
        for c in range(NCORES):
            nodes = np.flatnonzero(node_core == c)
            ds = degsum[nodes]
            node_sub = np.full(len(nodes), -1, np.int64)
            node_pos = np.full(len(nodes), -1, np.int64)
            cnt = np.zeros(nsub, np.int64)
            load = np.zeros(nsub, np.int64)
            for i in np.argsort(-ds, kind="stable"):
                l2, c2 = load + ds[i], cnt + 1
                hard = (c2 > SUB_CAP_N) | (l2 > SUB_CAP_E)
                key = np.where(hard, 1e18,
                               np.maximum(l2 / soft_e, c2 / soft_n))
                b = int(np.argmin(key))
                if key[b] >= 1e17:
                    ok = False
                    break
                node_sub[i] = b
                node_pos[i] = cnt[b]
                cnt[b] += 1
                load[b] += ds[i]
            if not ok:
                break
            packs.append((nodes, node_sub, node_pos))
        if ok:
            break
        nsub += 4
    assert ok, "node packing failed"

    spc = nsub * 32                       # slots per core
    nslots = NCORES * spc
    assert nslots - B_BASE <= 32768, "B-stream index overflow"
    node_slot = np.full(N, -1, np.int64)
    for c, (nodes, nsubv, nposv) in enumerate(packs):
        node_slot[nodes] = c * spc + nsubv * 32 + nposv

    # piece-major table row: row(c, p, loc) = 8*prow0[p] + c*prows[p]
    #                                           + (loc_slot - prow0[p])
    groups0 = _groups_of(nsub)
    gwr = [sg * 32 for sg in groups0]
    grow0 = np.cumsum([0] + gwr)[:-1]
    pieces = _pieces_of(groups0)
    prow0 = np.array([int(grow0[p0]) for p0, p1 in pieces])
    prows = np.array([int(sum(gwr[p0:p1])) for p0, p1 in pieces])
    sub0s = np.cumsum([0] + groups0)[:-1]
    sub2g = np.zeros(nsub, np.int64)
    for g, s0 in enumerate(sub0s):
        sub2g[s0:s0 + groups0[g]] = g
    g2p = np.zeros(len(groups0), np.int64)
    for pi, (p0, p1) in enumerate(pieces):
        g2p[p0:p1] = pi
    loc_slot = node_slot % spc
    n_p = g2p[sub2g[loc_slot // 32]]
    node_trow = (8 * prow0[n_p]
                 + (node_slot // spc) * prows[n_p]
                 + loc_slot - prow0[n_p])

    # per-edge info
    e_core = node_core[dst]
    e_sub = (node_slot[dst] % spc) // 32
    e_col = bond * 32 + (node_slot[dst] % 32)
    e_srcslot = node_trow[src]

    groups = groups0
    nchunks = nsub * 4
    gidx_cols = 32 * nsub                 # per group: 2 * subs * 16 cols
    gidx = np.zeros((NCORES, 128, gidx_cols), np.int16)
    dstl = np.full((NCORES, 128, nchunks), PAD_COL, np.float32)

    # group bases
    gsub0 = np.cumsum([0] + groups)[:-1]          # first sub of group
    gchunk0 = [int(s0 * 4) for s0 in gsub0]       # first chunk of group
    gcol0 = [int(s0 * 32) for s0 in gsub0]        # first gidx col of group

    for c in range(NCORES):
        esel = np.flatnonzero(e_core == c)
        sub = e_sub[esel]
        order_e = np.argsort(sub, kind="stable")
        es, subs_sorted = esel[order_e], sub[order_e]
        starts = np.searchsorted(subs_sorted, np.arange(nsub))
        ends = np.searchsorted(subs_sorted, np.arange(nsub), side="right")
        for g, subs_g in enumerate(groups):
            # linear idx lists per group: [subs_g * 256] rows per stream
            A_lin = np.zeros(subs_g * 256, np.int64)
            B_lin = np.zeros(subs_g * 256, np.int64)
            for sl in range(subs_g):
                s = int(gsub0[g]) + sl
                e_seg = es[starts[s]:ends[s]]
                slots = e_srcslot[e_seg]
                isA_must = slots < B_BASE
                isB_must = slots >= A_MAX
                band = ~(isA_must | isB_must)
                a_cnt = int(isA_must.sum())
                take_band_a = min(max(0, 256 - a_cnt), int(band.sum()))
                band_idx = np.flatnonzero(band)
                a_sel = np.concatenate(
                    [np.flatnonzero(isA_must), band_idx[:take_band_a]])
                b_sel = np.concatenate(
                    [np.flatnonzero(isB_must), band_idx[take_band_a:]])
                assert len(a_sel) <= 256 and len(b_sel) <= 256, \
                    f"A/B split overflow sub {s}: {len(a_sel)} {len(b_sel)}"
                for sel, lin, base in ((a_sel, A_lin, 0),
                                       (b_sel, B_lin, B_BASE)):
                    rows = slots[sel] - base
                    cols = e_col[e_seg[sel]]
                    k = np.arange(len(sel))
                    lin[sl * 256 + k] = rows
                    # chunk layout in group: A chunks [0, 2*subs_g),
                    # B chunks [2*subs_g, 4*subs_g); 2 chunks per sub/stream
                    ch_off = (0 if base == 0 else 2 * subs_g) + 2 * sl
                    dstl[c, k % 128,
                         gchunk0[g] + ch_off + k // 128] = cols
            # wrap: [16, 16*subs_g] tiled to 128 partitions
            half = subs_g * 16
            for hoff, lin in ((0, A_lin), (half, B_lin)):
                w = lin.reshape(subs_g * 16, 16).T    # [16, subs*16]
                gidx[c, :, gcol0[g] + hoff:gcol0[g] + hoff + half] = \
                    np.tile(w, (8, 1))
    return dict(node_slot=node_slot, nsub=nsub, spc=spc, nslots=nslots,
                gidx=gidx, dstl=dstl, groups=tuple(groups))


# ------------------------------------------------------------- bass graph --
def _build(nsub, spc, nslots, groups):
    nc = bass.Bass(target_bir_lowering=False, debug=False)
    nchunks = nsub * 4
    gidx_cols = 32 * nsub
    gsub0 = np.cumsum([0] + list(groups))[:-1]

    xT = nc.declare_dram_parameter("xT", [IN + 1, spc], F32, isOutput=False)
    lwT = nc.declare_dram_parameter("lwT", [IN + 1, 128], F32, isOutput=False)
    gidx = nc.declare_dram_parameter("gidx", [128, gidx_cols], I16,
                                     isOutput=False)
    dstl = nc.declare_dram_parameter("dstl", [128, nchunks], BF16,
                                     isOutput=False)
    # consts bf16: iota(128) | identity(128) | w_ihT(384) | w_hhT(384) |
    #              WtT2 (12*128)
    CCOLS = 128 + 128 + 384 + 384 + 12 * 128
    consts = nc.declare_dram_parameter("consts", [128, CCOLS], BF16,
                                       isOutput=False)
    mulvT = nc.declare_dram_parameter("mulvT", [128, 256], F32, isOutput=False)
    biases = nc.declare_dram_parameter("biases", [128, 8], F32, isOutput=False)
    muo = nc.declare_dram_parameter("muo", [128, spc], F32, isOutput=True)
    lvo = nc.declare_dram_parameter("lvo", [128, spc], F32, isOutput=True)

    gwr = [sg * 32 for sg in groups]
    grow0 = np.cumsum([0] + gwr)[:-1]
    pieces = _pieces_of(groups)
    piece_of_group = {}
    for pi, (p0, p1) in enumerate(pieces):
        for g in range(p0, p1):
            piece_of_group[g] = pi
    prow0 = [int(grow0[p0]) for p0, p1 in pieces]
    prows = [int(sum(gwr[p0:p1])) for p0, p1 in pieces]
    m_shards = [nc.dram_tensor(f"m_shard{p}", [prows[p], 128], BF16,
                               kind="Internal")
                for p in range(len(pieces))]
    m_tables = [nc.dram_tensor(f"m_table{i}", [nslots, 128], BF16,
                               kind="Internal", addr_space="Shared")
                for i in range(2)]

    with tile.TileContext(nc) as tc:
        with (
            tc.tile_pool(name="const", bufs=1) as cpool,
            tc.tile_pool(name="sb", bufs=2) as sb,
            tc.tile_pool(name="mg", bufs=2) as mgp,
            tc.tile_pool(name="oh", bufs=2) as ohp,
            tc.tile_pool(name="pt", bufs=2, space="PSUM") as ptp,
            tc.tile_pool(name="agg", bufs=1, space="PSUM") as aggp,
            tc.tile_pool(name="gate", bufs=1, space="PSUM") as gatep,
            tc.tile_pool(name="misc", bufs=2, space="PSUM") as miscp,
        ):
            nc.gpsimd.load_library(library_config.mlp)
            nregs = {}
            for subs_g in sorted(set(groups)):
                nregs[subs_g] = nc.gpsimd.to_reg(subs_g * 256)

            cst = cpool.tile([128, CCOLS], BF16)
            nc.sync.dma_start(cst[:], consts[:, :])
            iota = cst[:, 0:128]
            ident = cst[:, 128:256]
            wihT = cst[:, 256:640]
            whhT = cst[:, 640:1024]

            def WtT2(step, t):
                o = 1024 + (step * 4 + t) * 128
                return cst[:, o:o + 128]

            def coll_piece(tbl, p):
                r0 = 8 * prow0[p]
                nc.gpsimd.collective_compute(
                    "AllGather", mybir.AluOpType.bypass,
                    replica_groups=[list(range(NCORES))],
                    ins=[m_shards[p].ap().opt()],
                    outs=[tbl[r0:r0 + 8 * prows[p], :].opt()])

            mulv_sb = cpool.tile([128, 256], F32)
            nc.sync.dma_start(mulv_sb[:], mulvT[:, :])
            bia = cpool.tile([128, 8], F32)
            nc.sync.dma_start(bia[:], biases[:, :])
            gidx_sb = cpool.tile([128, gidx_cols], I16)
            nc.sync.dma_start(gidx_sb[:], gidx[:, :])
            dstl_sb = cpool.tile([128, nchunks], BF16)
            nc.sync.dma_start(dstl_sb[:], dstl[:, :])
            xT_sb = cpool.tile([IN + 1, spc], F32)
            nc.sync.dma_start(xT_sb[:], xT[:, :])
            lwT_sb = cpool.tile([IN + 1, 128], F32)
            nc.sync.dma_start(lwT_sb[:], lwT[:, :])

            # ---- m0 node-major -> per-group shards (collective ASAP) ----
            gsub0l = np.cumsum([0] + list(groups))[:-1]
            for g, subs_g in enumerate(groups):
                p = piece_of_group[g]
                goff = int(gsub0l[g]) * 32 - prow0[p]
                for mloc in range(subs_g // 4):
                    mb = int(gsub0l[g]) // 4 + mloc
                    sl = slice(mb * 128, (mb + 1) * 128)
                    psf = ptp.tile([128, 512], F32, tag="pt")
                    ps = psf[:, 0:128]
                    nc.tensor.matmul(out=ps, lhsT=xT_sb[:, sl],
                                     rhs=lwT_sb[:], start=True, stop=True)
                    mb_sb = sb.tile([128, 128], BF16, tag="m0s")
                    nc.scalar.activation(mb_sb[:], ps,
                                         mybir.ActivationFunctionType.Relu)
                    nc.sync.dma_start(
                        m_shards[p][goff + mloc * 128:
                                    goff + (mloc + 1) * 128, :],
                        mb_sb[:])
                for pi, (p0, p1) in enumerate(pieces):
                    if p1 - 1 == g:
                        coll_piece(m_tables[0], pi)

            # ---- x0 (overlaps with the first table's collectives) ----
            x0T_bf = cpool.tile([128, spc], BF16)
            x0T_f = cpool.tile([128, spc], F32)
            for i in range(0, spc, 512):
                w = min(512, spc - i)
                ps = ptp.tile([128, 512], F32, tag="pt")
                nc.tensor.matmul(out=ps[:, :w], lhsT=lwT_sb[:],
                                 rhs=xT_sb[:, i:i + w], start=True, stop=True)
                nc.scalar.activation(x0T_f[:, i:i + w], ps[:, :w],
                                     mybir.ActivationFunctionType.Relu)
                nc.vector.tensor_copy(x0T_bf[:, i:i + w], x0T_f[:, i:i + w])

            for step in range(STEPS):
                m_table = m_tables[step % 2]
                next_table = m_tables[(step + 1) % 2]

                for g, subs_g in enumerate(groups):
                    sub0 = int(gsub0[g])
                    c0 = sub0 * 4                    # first chunk of group
                    col0 = sub0 * 32                 # first gidx col
                    nch = subs_g * 4
                    half = subs_g * 16               # gidx cols per stream
                    mg = mgp.tile([128, 64, 128], BF16, tag="mg")
                    nc.gpsimd.dma_gather(
                        out_ap=mg[:, 0:2 * subs_g, :], in_ap=m_table[:, :],
                        idxs_ap=gidx_sb[:, col0:col0 + half],
                        num_idxs=subs_g * 256, num_idxs_reg=nregs[subs_g],
                        elem_size=128, single_packet=False)
                    nc.gpsimd.dma_gather(
                        out_ap=mg[:, 2 * subs_g:nch, :],
                        in_ap=m_table[B_BASE:, :],
                        idxs_ap=gidx_sb[:, col0 + half:col0 + 2 * half],
                        num_idxs=subs_g * 256, num_idxs_reg=nregs[subs_g],
                        elem_size=128, single_packet=False)
                    if step < STEPS - 1:
                        for pi, (p0, p1) in enumerate(pieces):
                            if p1 + 1 == g:
                                coll_piece(next_table, pi)
                    oh = ohp.tile([128, 64, 128], BF16, tag="oh")
                    dsl = dstl_sb[:, c0:c0 + nch]
                    nc.vector.tensor_tensor(
                        out=oh[:, 0:nch, :],
                        in0=dsl[:, :, None].to_broadcast([128, nch, 128]),
                        in1=iota[:, None, :].to_broadcast([128, nch, 128]),
                        op=mybir.AluOpType.is_equal)

                    # ---- scatter + transform per macro (4 subs) ----
                    agg = aggp.tile([128, 512], F32, tag="agg")
                    for mloc in range(subs_g // 4):
                        pt = ptp.tile([128, 512], F32, tag="pt")
                        for sl4 in range(4):
                            s_loc = mloc * 4 + sl4
                            reg = pt[:, sl4 * 128:(sl4 + 1) * 128]
                            for k in range(4):
                                ch = (2 * s_loc + k % 2) + \
                                    (2 * subs_g if k >= 2 else 0)
                                nc.tensor.matmul(
                                    out=reg, lhsT=mg[:, ch, :],
                                    rhs=oh[:, ch, :],
                                    start=(k == 0), stop=(k == 3))
                        pt_sb = sb.tile([128, 512], BF16, tag="pts")
                        nc.scalar.activation(
                            pt_sb[:], pt[:],
                            mybir.ActivationFunctionType.Copy)
                        for sl4 in range(4):
                            s_loc = mloc * 4 + sl4
                            cs = slice(mloc * 128 + sl4 * 32,
                                       mloc * 128 + (sl4 + 1) * 32)
                            for t in range(T):
                                nc.tensor.matmul(
                                    out=agg[:, cs], lhsT=WtT2(step, t),
                                    rhs=pt_sb[:, sl4 * 128 + t * 32:
                                              sl4 * 128 + (t + 1) * 32],
                                    start=(t == 0), stop=(t == 3))
                    gw = subs_g * 32                # slot width of group
                    agg_sb = sb.tile([128, 512], BF16, tag="aggs")
                    nc.scalar.activation(agg_sb[:, 0:gw], agg[:, 0:gw],
                                         mybir.ActivationFunctionType.Copy)

                    # ---- GRU per half-group (<=256 slots) ----
                    for h in range(-(-subs_g // 8)):
                        hw = min(256, gw - h * 256)
                        hsl = slice(h * 256, h * 256 + hw)
                        msl = slice(sub0 * 32 + h * 256,
                                    sub0 * 32 + h * 256 + hw)
                        GT = gatep.tile([128, 1024], F32, tag="GT")
                        for gi, wsl in ((0, slice(0, 128)),
                                        (1, slice(128, 256))):
                            out_sl = GT[:, gi * 256:gi * 256 + hw]
                            nc.tensor.matmul(out=out_sl,
                                             lhsT=wihT[:, wsl],
                                             rhs=agg_sb[:, hsl],
                                             start=True, stop=False)
                            nc.tensor.matmul(out=out_sl,
                                             lhsT=whhT[:, wsl],
                                             rhs=x0T_bf[:, msl],
                                             start=False, stop=True)
                        nc.tensor.matmul(out=GT[:, 512:512 + hw],
                                         lhsT=wihT[:, 256:384],
                                         rhs=agg_sb[:, hsl],
                                         start=True, stop=True)
                        nc.tensor.matmul(out=GT[:, 768:768 + hw],
                                         lhsT=whhT[:, 256:384],
                                         rhs=x0T_bf[:, msl],
                                         start=True, stop=True)
                        r_sb = sb.tile([128, 256], BF16, tag="r")
                        nc.scalar.activation(
                            r_sb[:, 0:hw], GT[:, 0:hw],
                            mybir.ActivationFunctionType.Sigmoid,
                            bias=bia[:, 0:1])
                        z_sb = sb.tile([128, 256], BF16, tag="z")
                        nc.scalar.activation(
                            z_sb[:, 0:hw], GT[:, 256:256 + hw],
                            mybir.ActivationFunctionType.Sigmoid,
                            bias=bia[:, 1:2])
                        ghn_sb = sb.tile([128, 256], F32, tag="ghn")
                        nc.scalar.activation(
                            ghn_sb[:, 0:hw], GT[:, 768:768 + hw],
                            mybir.ActivationFunctionType.Identity,
                            bias=bia[:, 3:4])
                        t1 = sb.tile([128, 256], F32, tag="t1")
                        nc.vector.tensor_tensor(out=t1[:, 0:hw],
                                                in0=r_sb[:, 0:hw],
                                                in1=ghn_sb[:, 0:hw],
                                                op=mybir.AluOpType.mult)
                        t2 = sb.tile([128, 256], F32, tag="t2")
                        nc.vector.tensor_tensor(out=t2[:, 0:hw],
                                                in0=t1[:, 0:hw],
                                                in1=GT[:, 512:512 + hw],
                                                op=mybir.AluOpType.add)
                        n_sb = sb.tile([128, 256], F32, tag="n")
                        nc.scalar.activation(
                            n_sb[:, 0:hw], t2[:, 0:hw],
                            mybir.ActivationFunctionType.Tanh,
                            bias=bia[:, 2:3])
                        d1 = sb.tile([128, 256], F32, tag="d1")
                        nc.vector.tensor_tensor(out=d1[:, 0:hw],
                                                in0=x0T_f[:, msl],
                                                in1=n_sb[:, 0:hw],
                                                op=mybir.AluOpType.subtract)
                        d2 = sb.tile([128, 256], F32, tag="d2")
                        nc.vector.tensor_tensor(out=d2[:, 0:hw],
                                                in0=z_sb[:, 0:hw],
                                                in1=d1[:, 0:hw],
                                                op=mybir.AluOpType.mult)
                        d3 = sb.tile([128, 256], F32, tag="d3")
                        nc.vector.tensor_tensor(out=d3[:, 0:hw],
                                                in0=n_sb[:, 0:hw],
                                                in1=d2[:, 0:hw],
                                                op=mybir.AluOpType.add)
                        if step < STEPS - 1:
                            mT_bf = sb.tile([128, 256], BF16, tag="mT")
                            nc.scalar.activation(
                                mT_bf[:, 0:hw], d3[:, 0:hw],
                                mybir.ActivationFunctionType.Relu)
                            for q in range(hw // 128):
                                qsl = slice(q * 128, (q + 1) * 128)
                                tp = miscp.tile([128, 128], BF16, tag="lp2")
                                nc.tensor.transpose(out=tp[:],
                                                    in_=mT_bf[:, qsl],
                                                    identity=ident)
                                m_sb = sb.tile([128, 128], BF16, tag="ms")
                                nc.vector.tensor_copy(m_sb[:], tp[:])
                                row0 = (sub0 * 32 + h * 256 + q * 128
                                        - prow0[piece_of_group[g]])
                                nc.sync.dma_start(
                                    m_shards[piece_of_group[g]][
                                        row0:row0 + 128, :],
                                    m_sb[:])
                        else:
                            mT_f = sb.tile([128, 256], F32, tag="mTf")
                            nc.scalar.activation(
                                mT_f[:, 0:hw], d3[:, 0:hw],
                                mybir.ActivationFunctionType.Relu)
                            for oi, (wsl, bsl, out_t) in enumerate((
                                    (mulv_sb[:, 0:128], bia[:, 4:5], muo),
                                    (mulv_sb[:, 128:256], bia[:, 5:6], lvo))):
                                psf = ptp.tile([128, 512], F32, tag="pt")
                                ps = psf[:, 0:hw]
                                nc.tensor.matmul(out=ps, lhsT=wsl,
                                                 rhs=mT_f[:, 0:hw],
                                                 start=True, stop=True)
                                o_sb = sb.tile([128, 256], F32, tag="osb")
                                nc.scalar.activation(
                                    o_sb[:, 0:hw], ps,
                                    mybir.ActivationFunctionType.Identity,
                                    bias=bsl)
                                nc.sync.dma_start(out_t[:, msl],
                                                  o_sb[:, 0:hw])
                if step < STEPS - 1:
                    for pi, (p0, p1) in enumerate(pieces):
                        if p1 + 1 >= len(groups):
                            coll_piece(next_table, pi)
    return nc


_CACHE = {}


def kernel(**inputs):
    _install_ntff_hook()
    pp = _preprocess(inputs["edge_index"], inputs["edge_attr"])
    nsub, spc, nslots = pp["nsub"], pp["spc"], pp["nslots"]
    groups = pp["groups"]
    node_slot = pp["node_slot"]

    x = np.asarray(inputs["x"], np.float32)
    lin_w = np.asarray(inputs["lin_w"], np.float32)
    lin_b = np.asarray(inputs["lin_b"], np.float32)
    gnn_w = np.asarray(inputs["gnn_w"], np.float32)
    w_ih = np.asarray(inputs["w_ih"], np.float32)
    w_hh = np.asarray(inputs["w_hh"], np.float32)
    b_ih = np.asarray(inputs["b_ih"], np.float32)
    b_hh = np.asarray(inputs["b_hh"], np.float32)
    mu_w = np.asarray(inputs["mu_w"], np.float32)
    mu_b = np.asarray(inputs["mu_b"], np.float32)
    lv_w = np.asarray(inputs["lv_w"], np.float32)
    lv_b = np.asarray(inputs["lv_b"], np.float32)

    # slotted x^T with ones row (bias via augmented matmul)
    x_slot = np.zeros((nslots, IN), np.float32)
    x_slot[node_slot] = x
    lwT_aug = np.concatenate([lin_w.T, lin_b[None, :]], 0).astype(np.float32)

    iota_t = np.tile(np.arange(128, dtype=np.float32)[None, :], (128, 1))
    ident = np.eye(128, dtype=np.float32)
    consts = np.concatenate([
        iota_t, ident, w_ih.T, w_hh.T,
        np.concatenate([(2.0 * gnn_w[s, t]).T for s in range(STEPS)
                        for t in range(T)], axis=1),
    ], axis=1).astype(BF)
    mulvT = np.concatenate([mu_w.T, lv_w.T], 1).astype(np.float32)
    biases = np.zeros((128, 8), np.float32)
    biases[:, 0] = b_ih[0:128] + b_hh[0:128]
    biases[:, 1] = b_ih[128:256] + b_hh[128:256]
    biases[:, 2] = b_ih[256:384]
    biases[:, 3] = b_hh[256:384]
    biases[:, 4] = mu_b
    biases[:, 5] = lv_b

    in_maps = []
    for c in range(NCORES):
        xs = x_slot[c * spc:(c + 1) * spc]
        xT_aug = np.concatenate([xs.T, np.ones((1, spc), np.float32)], 0)
        in_maps.append(dict(
            xT=xT_aug.astype(np.float32), lwT=lwT_aug,
            gidx=pp["gidx"][c], dstl=pp["dstl"][c].astype(BF),
            consts=consts, mulvT=mulvT, biases=biases))

    key = (nsub, spc, nslots, groups)
    if key not in _CACHE:
        nc = _build(nsub, spc, nslots, groups)
        mybir.codegen_inst_isa_subclasses(nc)
        _fix_sync_waits(nc)
        _CACHE[key] = nc
    nc = _CACHE[key]

    kernel.last_in_maps = in_maps
    kernel.last_node_slot = node_slot
    kernel.last_spc = spc
    res = run_bass_kernel_spmd(nc, in_maps, core_ids=list(range(NCORES)))
    kernel.last_results = res

    mu = np.zeros((N, L), np.float32)
    lv = np.zeros((N, L), np.float32)
    slot_core = node_slot // spc
    slot_loc = node_slot % spc
    for c in range(NCORES):
        sel = np.flatnonzero(slot_core == c)
        mu[sel] = res.results[c]["muo"][:, slot_loc[sel]].T
        lv[sel] = res.results[c]["lvo"][:, slot_loc[sel]].T
    return mu, lv
